# revision 9
# baseline (speedup 1.0000x reference)
"""CharRNN Trainium2 kernel — v4: v3 + xw PSUM-prefill (4 matmuls/step).

On top of v3: the per-step one-hot (emb_proj) matmuls leave the scan too.
All xw_t = emb_proj[x_t] are computed up front by batched one-hot matmuls
into a state ring buffer (xwT(l) is overwritten by hT(l+1), so no extra
SBUF).  During the scan, VectorE pre-fills each step's PSUM tile with xw
and the W_hh matmuls accumulate onto it with start=False — valid because
the slot's previous occupant's matmuls left the has_written bits set (a
one-time dummy matmul per PSUM slot primes the first uses).  The scan is
then 4 LDWEIGHTS+MATMUL pairs per step.


Improvements over v2 (driven by NTFF profile: TensorE 95.7% busy, pair rate
limited by per-matmul LDWEIGHTS ~120ns):
  * Logits leave the scan loop: hidden states accumulate in a persistent
    SBUF buffer hT_all [128, L*64] fp16; logits are computed afterwards as
    64 large-N (512) matmuls with strided access patterns over hT_all —
    ~17us instead of ~350us of per-step logits LDW+MM pairs.
  * b_out enters as a per-partition ScalarE bias (free) in the transposed
    logits layout; the host inverts the layout.
  * One 32-row chain per core (no batch chunks -> half the weight reloads);
    the tanh is split into h-halves writing to two separate PSUM banks so
    the half-A tanh overlaps the half-B matmuls of the same step.
"""

import os
import sys

import numpy as np

for _p in ("/opt/trn_rl_repo",):
    if _p not in sys.path and os.path.isdir(_p):
        sys.path.insert(0, _p)

import concourse.bass as bass
import concourse.mybir as mybir
from concourse import tile

B, L, V, E, H = 256, 512, 32, 64, 256
NCORES = 8
BL = B // NCORES  # 32 rows per core
F32 = mybir.dt.float32
F16 = mybir.dt.float16
TANH = mybir.ActivationFunctionType.Tanh
IDENT = mybir.ActivationFunctionType.Identity
LG_CHUNK = 16  # timesteps per logits matmul (N = LG_CHUNK*BL = 512)


def _split_excess_waits(nc, limit=1):
    """walrus (this pipeline) accepts at most ONE sync wait per instruction;
    move excess waits onto same-engine InstEventSemaphore carriers placed
    immediately before the over-limit instruction."""
    n_split = 0
    for bb in nc.main_func.blocks:
        out = []
        for ins in bb.instructions:
            si = getattr(ins, "sync_info", None)
            if si is not None and si.on_wait and len(si.on_wait) > limit:
                extra, keep = si.on_wait[:-limit], si.on_wait[-limit:]
                for j, w in enumerate(extra):
                    ev = mybir.InstEventSemaphore(
                        name=f"{ins.name}_wsplit{j}", ins=[], outs=[]
                    )
                    ev.engine = ins.engine
                    ev.sync_info = mybir.SyncInfo(on_wait=[w], on_update=[])
                    out.append(ev)
                    n_split += 1
                ins.sync_info = mybir.SyncInfo(
                    on_wait=list(keep), on_update=list(si.on_update)
                )
            out.append(ins)
        bb.instructions[:] = out
    return n_split



def _drop_redundant_waits(nc):
    """Vector-clock transitive reduction of semaphore waits.

    The scheduled program (one basic block, no control flow) is a valid
    linearization of the dependency order.  Replay it, tracking for each
    engine the set of (sem, value) facts it has observed — directly via its
    own waits/updates and transitively via the producer's knowledge snapshot
    at the waited value.  A wait already implied by the engine's current
    knowledge is deleted.  Purely removes waits; updates are untouched.
    """
    import collections

    # A (sem, value) fact is only eternally true for monotonic sems; barrier
    # sems get decremented and reused, so never drop waits on those.
    monotonic = {}
    for bb in nc.main_func.blocks:
        for ins in bb.instructions:
            si = getattr(ins, "sync_info", None)
            if si is None:
                continue
            for u in si.on_update:
                ok = (
                    u.update_mode in ("sem-inc", "sem-add-imm")
                    and u.update_reg is None
                )
                monotonic[u.id] = monotonic.get(u.id, True) and ok

    sem_val = collections.Counter()          # running value per sem id
    sem_hist = collections.defaultdict(list)  # sem id -> [(value, snapshot)]
    eng_vc = collections.defaultdict(dict)    # engine -> {sem id: value}
    dropped = 0

    def join(dst, src):
        for s, v in src.items():
            if dst.get(s, -1) < v:
                dst[s] = v

    for bb in nc.main_func.blocks:
        for ins in bb.instructions:
            eng = ins.engine
            vc = eng_vc[eng]
            si = getattr(ins, "sync_info", None)
            if si is None:
                continue
            if si.on_wait:
                def snap_of(w):
                    if (
                        w.wait_mode != "sem-ge-imm"
                        or w.wait_reg is not None
                        or not monotonic.get(w.id, False)
                    ):
                        return None
                    for val, snap in sem_hist[w.id]:
                        if val >= w.wait_value:
                            return snap
                    return {}

                snaps = [snap_of(w) for w in si.on_wait]
                remaining = list(range(len(si.on_wait)))
                for i, w in enumerate(si.on_wait):
                    if snaps[i] is None:
                        continue  # non-ge wait: never dropped
                    cover = dict(vc)
                    for j in remaining:
                        if j != i and snaps[j] is not None:
                            join(cover, snaps[j])
                    if cover.get(w.id, -1) >= w.wait_value:
                        remaining.remove(i)
                        dropped += 1
                kept = [si.on_wait[j] for j in remaining]
                for j in remaining:
                    w = si.on_wait[j]
                    if snaps[j] is not None:
                        join(vc, snaps[j])
                        if vc.get(w.id, -1) < w.wait_value:
                            vc[w.id] = w.wait_value
                if len(kept) != len(si.on_wait):
                    ins.sync_info = mybir.SyncInfo(
                        on_wait=kept, on_update=list(si.on_update)
                    )
            si = ins.sync_info
            for u in si.on_update:
                if u.update_mode in ("sem-inc", "sem-add-imm") and u.update_reg is None:
                    sem_val[u.id] += u.update_value
                    nv = sem_val[u.id]
                    vc[u.id] = max(vc.get(u.id, -1), nv)
                    sem_hist[u.id].append(
                        (nv, {k: v for k, v in vc.items() if monotonic.get(k, False)})
                    )
                elif u.update_mode in ("sem-dec", "sem-sub-imm") and u.update_reg is None:
                    sem_val[u.id] -= u.update_value
                else:
                    # unknown update (register/write): poison this sem's
                    # history so later waits on it are never dropped
                    sem_hist[u.id].append((10**12, {}))
                    sem_val[u.id] = 0
    return dropped


def build_bass(L=L):
    nc = bass.Bass(trn_type="TRN2")

    oh_d = nc.dram_tensor("oh", [V, L * BL], F16, kind="ExternalInput")
    # p32 cols: embT 0:32 | wih 32:288 | bh 288:544 | bout_col 544:545
    # p16 cols: wq 0:512 | wout 512:576 | h0T 576:640
    p32_d = nc.dram_tensor("p32", [128, 545], F32, kind="ExternalInput")
    p16_d = nc.dram_tensor("p16", [128, 640], F16, kind="ExternalInput")

    # transposed logits [v, l*BL + b] and final hT [128, 2*BL]
    logitsT_d = nc.dram_tensor("logitsT", [V, L * BL], F32, kind="ExternalOutput")
    hT_d = nc.dram_tensor("hT_out", [128, 2 * BL], F32, kind="ExternalOutput")

    with tile.TileContext(nc) as tc:
        with (
            tc.tile_pool(name="stage", bufs=1) as stage_pool,
            tc.tile_pool(name="const", bufs=1) as const,
            tc.tile_pool(name="ps0", bufs=3, space="PSUM") as ps0_pool,
            tc.tile_pool(name="ps1", bufs=3, space="PSUM") as ps1_pool,
            tc.tile_pool(name="pslg", bufs=2, space="PSUM") as pslg_pool,
        ):
            dve_chain = []

            def chain(inst):
                if dve_chain:
                    tile.add_dep_helper(
                        inst.ins, dve_chain[-1].ins, sync=False,
                        reason="prologue DVE order",
                    )
                dve_chain.append(inst)
                return inst

            st32 = stage_pool.tile([128, 545], F32, tag="st_p32")
            nc.sync.dma_start(st32[:], p32_d[:])
            st16 = stage_pool.tile([128, 640], F16, tag="st_p16")
            nc.sync.dma_start(st16[:], p16_d[:])

            def unpack(st, rows, c0, c1, dtype, name):
                dst = const.tile([rows, c1 - c0], dtype, tag=name)
                chain(nc.vector.tensor_copy(dst[:], st[0:rows, c0:c1]))
                return dst

            embT_sb = unpack(st32, E, 0, 32, F32, "c_embT")
            wih_sb = unpack(st32, E, 32, 288, F32, "c_wih")
            bh_sb = unpack(st32, 1, 288, 544, F32, "c_bh")
            bout_sb = unpack(st32, V, 544, 545, F32, "c_bout")  # column [V,1]
            ones_sb = const.tile([1, V], F32)
            chain(nc.vector.memset(ones_sb[:], 1.0))
            embP_sb = const.tile([V, H], F16)

            # emb_proj = embedding @ W_ih + b_h (fp32 compute, fp16 store)
            ps_e = pslg_pool.tile([V, H], F32, tag="lgT")
            nc.tensor.matmul(ps_e[:], embT_sb[:], wih_sb[:], start=True, stop=False)
            nc.tensor.matmul(ps_e[:], ones_sb[:], bh_sb[:], start=False, stop=True)
            chain(nc.vector.tensor_copy(embP_sb[:], ps_e[:]))

            wq_sb = unpack(st16, 128, 0, 512, F16, "c_wq")
            wout_sb = unpack(st16, 128, 512, 576, F16, "c_wout")

            oh_sb = const.tile([V, L * BL], F16)
            nc.sync.dma_start(oh_sb[:], oh_d[:])

            # state ring: cols l*64 + [0:32, 32:64].  Slot 0 = h0; slot
            # l+1 first holds xwT(l) (from the precompute phase), which the
            # scan consumes at step l and replaces with hT(l+1).
            W = 2 * BL  # 64 cols per step
            hT_all = const.tile([128, (L + 1) * W], F16)
            chain(nc.vector.tensor_copy(hT_all[:, 0:W], st16[:, 576:640]))

            # ---- xw precompute: xwT(l) = emb_proj.T @ onehot(l) ----------
            # batched over PRE timesteps per matmul pair (N = PRE*BL),
            # strided output into the ring's slots l+1
            PRE = 16

            def emit_precompute(c):
                rhs = oh_sb[:, c * PRE * BL : (c + 1) * PRE * BL]
                out_view = hT_all[
                    :, (1 + c * PRE) * W : (1 + (c + 1) * PRE) * W
                ].rearrange("p (l c) -> p l c", c=W)
                ps_x = pslg_pool.tile([128, PRE * BL], F32, tag="lgT")
                nc.tensor.matmul(ps_x[:], embP_sb[:, 0:128], rhs,
                                 start=True, stop=True)
                nc.vector.tensor_copy(
                    out_view[:, :, 0:BL],
                    ps_x[:].rearrange("p (l c) -> p l c", c=BL),
                )
                ps_y = pslg_pool.tile([128, PRE * BL], F32, tag="lgT")
                nc.tensor.matmul(ps_y[:], embP_sb[:, 128:256], rhs,
                                 start=True, stop=True)
                nc.vector.tensor_copy(
                    out_view[:, :, BL : 2 * BL],
                    ps_y[:].rearrange("p (l c) -> p l c", c=BL),
                )

            logitsT_sb = const.tile([V, L * BL], F32)

            def emit_logits(c):
                base = (1 + c * LG_CHUNK) * W
                view = hT_all[:, base : base + LG_CHUNK * W].rearrange(
                    "p (l c) -> p l c", c=W
                )
                lgT = pslg_pool.tile([V, LG_CHUNK * BL], F32, tag="lgT")
                nc.tensor.matmul(
                    lgT[:], wout_sb[:, 0:V], view[:, :, 0:BL],
                    start=True, stop=False,
                )
                nc.tensor.matmul(
                    lgT[:], wout_sb[:, V : 2 * V], view[:, :, BL : 2 * BL],
                    start=False, stop=True,
                )
                nc.vector.tensor_scalar_add(
                    logitsT_sb[:, c * LG_CHUNK * BL : (c + 1) * LG_CHUNK * BL],
                    lgT[:],
                    bout_sb[:],
                )

            # xw for the first steps must exist before the scan starts
            for c0 in range(min(2, L // PRE)):
                emit_precompute(c0)

            # ---- prime PSUM slots (set has_written bits once) ------------
            zrow_sb = const.tile([1, 128], F16)
            chain(nc.vector.memset(zrow_sb[:], 0.0))
            zbl_sb = const.tile([1, BL], F16)
            chain(nc.vector.memset(zbl_sb[:], 0.0))
            primed = []
            for pool in (ps0_pool, ps1_pool):
                tag = "ps0" if pool is ps0_pool else "ps1"
                for _ in range(3):
                    pz = pool.tile([128, BL], F32, tag=tag)
                    nc.tensor.matmul(
                        pz[:], zrow_sb[:], zbl_sb[:], start=True, stop=True
                    )
                    primed.append(pz)

            # ---- scan: psum prefilled with xw, W matmuls accumulate ------
            prev_tanh = [None, None]
            for l in range(L):
                h0c = slice(l * W, l * W + BL)
                h1c = slice(l * W + BL, l * W + W)
                x0c = slice((l + 1) * W, (l + 1) * W + BL)       # xw half0
                x1c = slice((l + 1) * W + BL, (l + 1) * W + W)   # xw half1

                ps0 = ps0_pool.tile([128, BL], F32, tag="ps0")
                # prefill gated only by slot release -> runs steps ahead;
                # the MM's resulting 2nd wait becomes a PE carrier that
                # executes inside the idle window (cheaper than serializing
                # the prefill behind the previous tanh)
                nc.vector.tensor_copy(ps0[:], hT_all[:, x0c])
                nc.tensor.matmul(
                    ps0[:], wq_sb[:, 0:128], hT_all[:, h0c],
                    start=False, stop=False, skip_group_check=True,
                )
                nc.tensor.matmul(
                    ps0[:], wq_sb[:, 128:256], hT_all[:, h1c],
                    start=False, stop=True, skip_group_check=True,
                )
                prev_tanh[0] = nc.scalar.activation(hT_all[:, x0c], ps0[:], TANH)

                ps1 = ps1_pool.tile([128, BL], F32, tag="ps1")
                nc.vector.tensor_copy(ps1[:], hT_all[:, x1c])
                nc.tensor.matmul(
                    ps1[:], wq_sb[:, 256:384], hT_all[:, h0c],
                    start=False, stop=False, skip_group_check=True,
                )
                nc.tensor.matmul(
                    ps1[:], wq_sb[:, 384:512], hT_all[:, h1c],
                    start=False, stop=True, skip_group_check=True,
                )
                prev_tanh[1] = nc.scalar.activation(hT_all[:, x1c], ps1[:], TANH)

                if (l + 1) % PRE == 0:
                    nxt = (l + 1) // PRE + 1
                    if nxt < L // PRE:
                        emit_precompute(nxt)
                if (l + 1) % LG_CHUNK == 0:
                    emit_logits((l + 1) // LG_CHUNK - 1)

            # (logits chunks are emitted interleaved into the scan loop)

            # final h in fp32
            hT_f32 = const.tile([128, 2 * BL], F32)
            nc.vector.tensor_copy(hT_f32[:], hT_all[:, L * W : (L + 1) * W])

            nc.sync.dma_start(logitsT_d[:], logitsT_sb[:])
            nc.sync.dma_start(hT_d[:], hT_f32[:])

    return nc


def make_in_maps(x, h0, embedding, W_ih, W_hh, b_h, W_out, b_out):
    x = np.asarray(x)
    h0 = np.asarray(h0, dtype=np.float32)
    embedding = np.asarray(embedding, dtype=np.float32)
    W_ih = np.asarray(W_ih, dtype=np.float32)
    W_hh = np.asarray(W_hh, dtype=np.float32)
    b_h = np.asarray(b_h, dtype=np.float32)
    W_out = np.asarray(W_out, dtype=np.float32)
    b_out = np.asarray(b_out, dtype=np.float32)

    L_cur = x.shape[1]
    p32 = np.zeros((128, 545), np.float32)
    p32[0:E, 0:32] = embedding.T
    p32[0:E, 32:288] = W_ih
    p32[0, 288:544] = b_h
    p32[0:V, 544] = b_out
    p16 = np.zeros((128, 640), np.float16)
    p16[:, 0:512] = np.concatenate(
        [
            W_hh[0:128, 0:128],
            W_hh[128:256, 0:128],
            W_hh[0:128, 128:256],
            W_hh[128:256, 128:256],
        ],
        axis=1,
    ).astype(np.float16)
    p16[:, 512:576] = np.concatenate(
        [W_out[0:128, :], W_out[128:256, :]], axis=1
    ).astype(np.float16)

    in_maps = []
    for c in range(NCORES):
        xc = x[c * BL : (c + 1) * BL]
        oh = (np.arange(V, dtype=np.int32)[:, None, None] == xc.T[None, :, :])
        oh = np.ascontiguousarray(oh.reshape(V, L_cur * BL).astype(np.float16))
        h0c = h0[c * BL : (c + 1) * BL]
        p16c = p16.copy()
        p16c[:, 576:608] = h0c[:, 0:128].T.astype(np.float16)
        p16c[:, 608:640] = h0c[:, 128:256].T.astype(np.float16)
        in_maps.append({"p32": p32, "p16": p16c, "oh": oh})
    return in_maps


def assemble_outputs(results):
    logits = np.empty((B, L, V), dtype=np.float32)
    h_final = np.empty((B, H), dtype=np.float32)
    for c, res in enumerate(results):
        lgT = res["logitsT"].reshape(V, L, BL)  # [v, l, b]
        logits[c * BL : (c + 1) * BL] = np.transpose(lgT, (2, 1, 0))
        hT = res["hT_out"]
        h_final[c * BL : (c + 1) * BL, 0:128] = hT[:, 0:BL].T
        h_final[c * BL : (c + 1) * BL, 128:256] = hT[:, BL : 2 * BL].T
    return logits, h_final


def kernel(x, h0, embedding, W_ih, W_hh, b_h, W_out, b_out, **kwargs):
    from concourse.bass_utils import run_bass_kernel_spmd

    nc = build_bass()
    _drop_redundant_waits(nc)
    _split_excess_waits(nc)
    in_maps = make_in_maps(x, h0, embedding, W_ih, W_hh, b_h, W_out, b_out)
    out = run_bass_kernel_spmd(nc, in_maps, core_ids=list(range(NCORES)))
    return assemble_outputs(out.results)
